# revision 20
# baseline (speedup 1.0000x reference)
"""LRNetLinear forward on 8 Trainium2 NeuronCores — fp8 DoubleRow tensor-parallel.

Host staging (format conversion only): x -> xT fp8 hi+lo planes; theta/eps/
scales -> bf16. Device: shift-free 3-way softmax prep (bf16 chain), fp8
hi/lo mean weights + fp8 var weights via PE transposes with per-half
evacuation, x2 = xh^2 split across ACT/DVE/Pool, 3 fp8 DR mu chains + 1 var
chain, j-major prep overlapped with partial-N chains for 3 open tiles.
"""
import sys

if "/opt/trn_rl_repo" not in sys.path:
    sys.path.insert(0, "/opt/trn_rl_repo")

import numpy as np

import concourse.bass as bass
import concourse.bacc as bacc
import concourse.mybir as mybir
import concourse.tile as tile
from concourse.bass_utils import run_bass_kernel_spmd
from concourse.masks import make_identity

N_CORES = 8
B = 4096
I = 4096
O = 4096
OS = O // N_CORES
KT = I // 128
BT = B // 128
GT = 4
NG = BT // GT
OJ = OS // 128
IC = 1024
NH = I // IC
KC = IC // 128
F32 = mybir.dt.float32
BF16 = mybir.dt.bfloat16
FP8 = mybir.dt.float8e4
PSUM = bass.MemorySpace.PSUM
DR = mybir.MatmulPerfMode.DoubleRow

SW = 64.0
SV = 4096.0
N_OPEN = 3
X2_ACT, X2_DVE = 16, 10

_CACHE = {}


def build():
    AF = mybir.ActivationFunctionType
    OP = mybir.AluOpType
    nc = bacc.Bacc("TRN2", target_bir_lowering=False, debug=False, num_devices=N_CORES)
    xh_d = nc.dram_tensor("xh", [I, B], FP8, kind="ExternalInput").ap()
    xl_d = nc.dram_tensor("xl", [I, B], FP8, kind="ExternalInput").ap()
    tn_d = nc.dram_tensor("tn", [OS, I], BF16, kind="ExternalInput").ap()
    tp_d = nc.dram_tensor("tp", [OS, I], BF16, kind="ExternalInput").ap()
    scs_d = nc.dram_tensor("scs", [OS, KT], BF16, kind="ExternalInput").ap()
    scs2_d = nc.dram_tensor("scs2", [OS, KT], BF16, kind="ExternalInput").ap()
    eps_d = nc.dram_tensor("eps", [B, OS], BF16, kind="ExternalInput").ap()
    out_d = nc.dram_tensor("out", [B, OS], F32, kind="ExternalOutput").ap()

    xh_r = xh_d.rearrange("(k p) b -> p k b", p=128)
    xl_r = xl_d.rearrange("(k p) b -> p k b", p=128)
    eps_r = eps_d.rearrange("(g t p) o -> p g t o", p=128, t=GT)
    out_r = out_d.rearrange("(g t p) o -> p g t o", p=128, t=GT)

    with tile.TileContext(nc) as tc:
        with tc.tile_pool(name="const", bufs=1) as cp:
            identb = cp.tile([128, 128], BF16, name="identb")
            b1e8 = cp.tile([128, 1], F32, name="b1e8")
            nc.vector.memset(b1e8, 1e-8)
            scs_t = cp.tile([128, OJ, KT], BF16, name="scs_t")
            nc.sync.dma_start(scs_t, scs_d.rearrange("(j p) k -> p j k", p=128))
            sc2s = cp.tile([128, OJ, KT], BF16, name="sc2s")
            nc.sync.dma_start(sc2s, scs2_d.rearrange("(j p) k -> p j k", p=128))
            whT = cp.tile([128, KT, OS], FP8, name="whT")
            wlT = cp.tile([128, KT, OS], FP8, name="wlT")
            wvT = cp.tile([128, KT, OS], FP8, name="wvT")

            with (
                tc.tile_pool(name="wprep", bufs=1) as wp,
                tc.tile_pool(name="wpsum", bufs=1, space=PSUM) as wps,
                tc.tile_pool(name="xg", bufs=1) as xgp,
                tc.tile_pool(name="eg", bufs=1) as egp,
                tc.tile_pool(name="og", bufs=1) as ogp,
                tc.tile_pool(name="mpsum", bufs=1, space=PSUM) as ops,
            ):
                ident = wp.tile([128, 128], F32, name="ident")
                make_identity(nc, ident)
                nc.vector.tensor_copy(identb, ident)

                groups = {}

                def load_group(g, piece=None):
                    gs = slice(512 * g, 512 * (g + 1))
                    KH = KT // 2
                    if piece in (None, 0):
                        xh_g = xgp.tile([128, KT, 512], FP8, tag="xh", bufs=2, name="xh_g")
                        xl_g = xgp.tile([128, KT, 512], FP8, tag="xl", bufs=2, name="xl_g")
                        eps_g = egp.tile([128, GT, OS], BF16, tag="eps", bufs=2, name="eps_g")
                        groups[g] = [xh_g, xl_g, eps_g, None, None]
                    xh_g, xl_g, eps_g = groups[g][:3]
                    if piece is None:
                        nc.sync.dma_start(xh_g, xh_r[:, :, gs])
                        nc.sync.dma_start(xl_g, xl_r[:, :, gs])
                        nc.sync.dma_start(eps_g, eps_r[:, g])
                    elif piece == 0:
                        nc.sync.dma_start(xh_g[:, :KH, :], xh_r[:, :KH, gs])
                    elif piece == 1:
                        nc.sync.dma_start(xh_g[:, KH:, :], xh_r[:, KH:, gs])
                    elif piece == 2:
                        nc.sync.dma_start(xl_g[:, :KH, :], xl_r[:, :KH, gs])
                        nc.sync.dma_start(eps_g, eps_r[:, g])
                    elif piece == 3:
                        nc.sync.dma_start(xl_g[:, KH:, :], xl_r[:, KH:, gs])

                def x2_group(g):
                    xh_g = groups[g][0]
                    x2_g = xgp.tile([128, KT, 512], FP8, tag="x2", bufs=2, name="x2_g")
                    a, b_ = X2_ACT, X2_ACT + X2_DVE
                    for k0_ in range(0, a, 4):
                        sl_ = slice(k0_, min(k0_ + 4, a))
                        nc.scalar.activation(x2_g[:, sl_, :], xh_g[:, sl_, :], AF.Square)
                    for k0_ in range(a, b_, 5):
                        sl_ = slice(k0_, min(k0_ + 5, b_))
                        nc.vector.tensor_tensor(x2_g[:, sl_, :], xh_g[:, sl_, :],
                                                xh_g[:, sl_, :], op=OP.mult)
                    for k0_ in range(b_, KT, 3):
                        sl_ = slice(k0_, min(k0_ + 3, KT))
                        nc.gpsimd.tensor_tensor(x2_g[:, sl_, :], xh_g[:, sl_, :],
                                                xh_g[:, sl_, :], op=OP.mult)
                    groups[g][3] = x2_g
                    outg = ogp.tile([128, GT, OS], F32, tag="og", bufs=1, name="outg")
                    groups[g][4] = outg

                def prep_unit(j, h):
                    i0 = h * IC
                    k0 = h * KC
                    js = slice(128 * j, 128 * (j + 1))
                    tn_t = wp.tile([128, IC], BF16, tag="tn", bufs=2, name="tn_t")
                    nc.sync.dma_start(tn_t, tn_d[js, i0:i0 + IC])
                    tp_t = wp.tile([128, IC], BF16, tag="tp", bufs=2, name="tp_t")
                    nc.sync.dma_start(tp_t, tp_d[js, i0:i0 + IC])
                    en = wp.tile([128, IC], BF16, tag="en", bufs=2, name="en")
                    nc.scalar.activation(en, tn_t, AF.Exp)
                    ep = wp.tile([128, IC], BF16, tag="ep", bufs=2, name="ep")
                    nc.scalar.activation(ep, tp_t, AF.Exp)
                    s1 = wp.tile([128, IC], BF16, tag="s1", bufs=1, name="s1")
                    nc.vector.scalar_tensor_tensor(s1, en, 1.0, ep,
                                                   op0=OP.add, op1=OP.add)
                    r = wp.tile([128, IC], BF16, tag="r", bufs=1, name="r")
                    with nc.allow_low_precision(reason="r in bf16 is within tolerance"):
                        nc.vector.reciprocal(r, s1)
                    d = wp.tile([128, IC], BF16, tag="d", bufs=1, name="d")
                    nc.vector.tensor_tensor(d, ep, en, op=OP.subtract)
                    diff = wp.tile([128, KC, 128], BF16, tag="diff", bufs=2, name="diff")
                    nc.vector.tensor_tensor(
                        diff, d.rearrange("p (k b) -> p k b", k=KC),
                        r.rearrange("p (k b) -> p k b", k=KC), op=OP.mult)
                    t2 = wp.tile([128, IC], BF16, tag="t2", bufs=1, name="t2")
                    nc.vector.tensor_tensor(t2, diff.rearrange("p k b -> p (k b)"),
                                            d, op=OP.mult)
                    t3 = wp.tile([128, IC], BF16, tag="t3", bufs=1, name="t3")
                    nc.vector.scalar_tensor_tensor(t3, en, -1.0, t2,
                                                   op0=OP.mult, op1=OP.add)
                    t4 = wp.tile([128, IC], BF16, tag="t4", bufs=1, name="t4")
                    nc.vector.tensor_tensor(t4, ep, t3, op=OP.subtract)
                    wvc = wp.tile([128, KC, 128], BF16, tag="wvc", bufs=2, name="wvc")
                    nc.vector.tensor_tensor(
                        wvc, t4.rearrange("p (k b) -> p k b", k=KC),
                        r.rearrange("p (k b) -> p k b", k=KC), op=OP.mult)
                    scb = scs_t[:, j, k0:k0 + KC, None].broadcast_to((128, KC, 128))
                    smw = wp.tile([128, KC, 128], BF16, tag="smw", bufs=2, name="smw")
                    nc.gpsimd.tensor_tensor(smw, diff, scb, op=OP.mult)
                    sc2b = sc2s[:, j, k0:k0 + KC, None].broadcast_to((128, KC, 128))
                    wv = wp.tile([128, KC, 128], BF16, tag="wv", bufs=2, name="wv")
                    nc.gpsimd.tensor_tensor(wv, wvc, sc2b, op=OP.mult)
                    # transpose + evacuate per half so the next unit's
                    # transposes only wait on half an evac round-trip
                    pmw = wps.tile([128, KC, 128], BF16, tag="pmw", name="pmw")
                    pwv = wps.tile([128, KC, 128], BF16, tag="pwv", name="pwv")
                    KH2 = KC // 2
                    for hf in range(2):
                        ps = slice(hf * KH2, (hf + 1) * KH2)
                        for kb in range(hf * KH2, (hf + 1) * KH2):
                            nc.tensor.transpose(pmw[:, kb, :], smw[:, kb, :], identb)
                            nc.tensor.transpose(pwv[:, kb, :], wv[:, kb, :], identb)
                        hs = slice(k0 + hf * KH2, k0 + (hf + 1) * KH2)
                        nc.scalar.activation(whT[:, hs, js], pmw[:, ps], AF.Copy)
                        nc.vector.tensor_tensor(wlT[:, hs, js], pmw[:, ps],
                                                whT[:, hs, js], op=OP.subtract)
                        nc.scalar.activation(wvT[:, hs, js], pwv[:, ps], AF.Copy)

                def mm(pt, lhs, rhs, st, sp):
                    nc.tensor.matmul(pt, lhs, rhs, start=st, stop=sp,
                                     perf_mode=DR, skip_group_check=True)

                def mu_chain(t, pmu, js):
                    g, tt = t // GT, t % GT
                    xh_g, xl_g = groups[g][0], groups[g][1]
                    ts = slice(128 * tt, 128 * (tt + 1))
                    KK = KT // 2
                    for kk in range(KK):
                        ks = slice(2 * kk, 2 * kk + 2)
                        mm(pmu[:, js], xh_g[:, ks, ts], whT[:, ks, js], kk == 0, False)
                        mm(pmu[:, js], xh_g[:, ks, ts], wlT[:, ks, js], False, False)
                        mm(pmu[:, js], xl_g[:, ks, ts], whT[:, ks, js], False, kk == KK - 1)

                def var_chain(t, pvar, js):
                    g, tt = t // GT, t % GT
                    x2_g = groups[g][3]
                    ts = slice(128 * tt, 128 * (tt + 1))
                    KK = KT // 2
                    for kk in range(KK):
                        ks = slice(2 * kk, 2 * kk + 2)
                        mm(pvar[:, js], x2_g[:, ks, ts], wvT[:, ks, js],
                           kk == 0, kk == KK - 1)

                def output_compute(t, pmu, pvar):
                    g, tt = t // GT, t % GT
                    eps_g = groups[g][2]
                    outg = groups[g][4]
                    sig = ogp.tile([128, OS], BF16, tag="sig", bufs=1, name="sig")
                    nc.scalar.activation(sig, pvar, AF.Sqrt, bias=b1e8, scale=1.0 / SV)
                    prod = ogp.tile([128, OS], BF16, tag="prod", bufs=1, name="prod")
                    nc.vector.tensor_tensor(prod, sig, eps_g[:, tt, :], op=OP.mult)
                    nc.vector.scalar_tensor_tensor(outg[:, tt, :], pmu, 1.0 / SW, prod,
                                                   op0=OP.mult, op1=OP.add)

                def out_dma(g):
                    nc.sync.dma_start(out_r[:, g], groups[g][4])

                def new_pm():
                    return ops.tile([128, OS], F32, tag="pm", bufs=3, name="pmu")

                def new_pv():
                    return ops.tile([128, OS], F32, tag="pv", bufs=3, name="pvar")

                # ---- schedule ----
                units = [(j, h) for j in range(OJ) for h in range(NH)]
                opened = {}
                full = slice(0, OS)
                for u, (j, h) in enumerate(units):
                    prep_unit(j, h)
                    if u < 4:
                        load_group(0, piece=u)
                    if u == 1:
                        x2_group(0)
                    if h == NH - 1:
                        if j == 0:
                            for t in range(N_OPEN):
                                opened[t] = [new_pm(), new_pv()]
                        js = slice(128 * j, 128 * (j + 1))
                        for t in range(N_OPEN):
                            mu_chain(t, opened[t][0], js)
                        for t in range(N_OPEN):
                            var_chain(t, opened[t][1], js)
                    if u == 13:
                        load_group(1)

                for t_ in range(N_OPEN):
                    output_compute(t_, *opened[t_])
                opened.clear()

                x2_group(1)
                pending = None
                for t in range(N_OPEN, BT):
                    g, tt = t // GT, t % GT
                    if pending is not None:
                        output_compute(*pending)
                        if pending[0] % GT == GT - 1:
                            out_dma(pending[0] // GT)
                        pending = None
                    pmu, pvar = new_pm(), new_pv()
                    var_chain(t, pvar, full)
                    mu_chain(t, pmu, full)
                    if tt == 0 and 1 <= g and g + 1 < NG:
                        load_group(g + 1)
                    if tt == 2 and 1 <= g and g + 1 < NG:
                        x2_group(g + 1)
                    pending = (t, pmu, pvar)
                output_compute(*pending)
                out_dma(NG - 1)

    nc.compile()
    return nc


def _get_nc():
    if "nc" not in _CACHE:
        _CACHE["nc"] = build()
    return _CACHE["nc"]


def kernel(x, theta_neg, theta_pos, scales_exp, eps):
    nc = _get_nc()
    np_fp8 = mybir.dt.np(FP8)
    np_bf16 = mybir.dt.np(BF16)
    xT = np.asarray(x, np.float32).T
    xh = np.ascontiguousarray(xT).astype(np_fp8)
    xl = (xT - xh.astype(np.float32)).astype(np_fp8)
    tn = np.asarray(theta_neg, np.float32).astype(np_bf16)
    tp = np.asarray(theta_pos, np.float32).astype(np_bf16)
    sc = np.asarray(scales_exp[:, ::128], np.float32)
    scs = (SW * sc).astype(np_bf16)
    scs2 = (SV * sc * sc).astype(np_bf16)
    epsb = np.asarray(eps, np.float32).astype(np_bf16)
    in_maps = []
    for j in range(N_CORES):
        sl = slice(OS * j, OS * (j + 1))
        in_maps.append({
            "xh": xh,
            "xl": xl,
            "tn": np.ascontiguousarray(tn[sl]),
            "tp": np.ascontiguousarray(tp[sl]),
            "scs": np.ascontiguousarray(scs[sl]),
            "scs2": np.ascontiguousarray(scs2[sl]),
            "eps": np.ascontiguousarray(epsb[:, sl]),
        })
    res = run_bass_kernel_spmd(nc, in_maps, core_ids=list(range(N_CORES)))
    return np.concatenate([res.results[j]["out"] for j in range(N_CORES)], axis=1)


# revision 21
# speedup vs baseline: 1.0082x; 1.0082x over previous
"""LRNetLinear forward on 8 Trainium2 NeuronCores — fp8 DoubleRow tensor-parallel.

Host staging (format conversion only): x -> xT fp8 hi+lo planes; theta/eps/
scales -> bf16. Device: shift-free 3-way softmax prep (bf16 chain), fp8
hi/lo mean weights + fp8 var weights via PE transposes with per-half
evacuation, x2 = xh^2 split across ACT/DVE/Pool, 3 fp8 DR mu chains + 1 var
chain, j-major prep overlapped with partial-N chains for 3 open tiles.
"""
import sys

if "/opt/trn_rl_repo" not in sys.path:
    sys.path.insert(0, "/opt/trn_rl_repo")

import numpy as np

import concourse.bass as bass
import concourse.bacc as bacc
import concourse.mybir as mybir
import concourse.tile as tile
from concourse.bass_utils import run_bass_kernel_spmd
from concourse.masks import make_identity

N_CORES = 8
B = 4096
I = 4096
O = 4096
OS = O // N_CORES
KT = I // 128
BT = B // 128
GT = 4
NG = BT // GT
OJ = OS // 128
IC = 1024
NH = I // IC
KC = IC // 128
F32 = mybir.dt.float32
BF16 = mybir.dt.bfloat16
FP8 = mybir.dt.float8e4
PSUM = bass.MemorySpace.PSUM
DR = mybir.MatmulPerfMode.DoubleRow

SW = 64.0
SV = 4096.0
N_OPEN = 3
X2_ACT, X2_DVE = 16, 10

_CACHE = {}


def build():
    AF = mybir.ActivationFunctionType
    OP = mybir.AluOpType
    nc = bacc.Bacc("TRN2", target_bir_lowering=False, debug=False, num_devices=N_CORES)
    xh_d = nc.dram_tensor("xh", [I, B], FP8, kind="ExternalInput").ap()
    xl_d = nc.dram_tensor("xl", [I, B], FP8, kind="ExternalInput").ap()
    tn_d = nc.dram_tensor("tn", [OS, I], BF16, kind="ExternalInput").ap()
    tp_d = nc.dram_tensor("tp", [OS, I], BF16, kind="ExternalInput").ap()
    scs_d = nc.dram_tensor("scs", [OS, KT], BF16, kind="ExternalInput").ap()
    scs2_d = nc.dram_tensor("scs2", [OS, KT], BF16, kind="ExternalInput").ap()
    eps_d = nc.dram_tensor("eps", [B, OS], BF16, kind="ExternalInput").ap()
    out_d = nc.dram_tensor("out", [B, OS], F32, kind="ExternalOutput").ap()

    xh_r = xh_d.rearrange("(k p) b -> p k b", p=128)
    xl_r = xl_d.rearrange("(k p) b -> p k b", p=128)
    eps_r = eps_d.rearrange("(g t p) o -> p g t o", p=128, t=GT)
    out_r = out_d.rearrange("(g t p) o -> p g t o", p=128, t=GT)

    with tile.TileContext(nc) as tc:
        with tc.tile_pool(name="const", bufs=1) as cp:
            identb = cp.tile([128, 128], BF16, name="identb")
            b1e8 = cp.tile([128, 1], F32, name="b1e8")
            nc.vector.memset(b1e8, 1e-8)
            scs_t = cp.tile([128, OJ, KT], BF16, name="scs_t")
            nc.sync.dma_start(scs_t, scs_d.rearrange("(j p) k -> p j k", p=128))
            sc2s = cp.tile([128, OJ, KT], BF16, name="sc2s")
            nc.sync.dma_start(sc2s, scs2_d.rearrange("(j p) k -> p j k", p=128))
            whT = cp.tile([128, KT, OS], FP8, name="whT")
            wlT = cp.tile([128, KT, OS], FP8, name="wlT")
            wvT = cp.tile([128, KT, OS], FP8, name="wvT")

            with (
                tc.tile_pool(name="wprep", bufs=1) as wp,
                tc.tile_pool(name="wpsum", bufs=1, space=PSUM) as wps,
                tc.tile_pool(name="xg", bufs=1) as xgp,
                tc.tile_pool(name="eg", bufs=1) as egp,
                tc.tile_pool(name="og", bufs=1) as ogp,
                tc.tile_pool(name="mpsum", bufs=1, space=PSUM) as ops,
            ):
                ident = wp.tile([128, 128], F32, name="ident")
                make_identity(nc, ident)
                nc.vector.tensor_copy(identb, ident)

                groups = {}

                def load_group(g, piece=None):
                    gs = slice(512 * g, 512 * (g + 1))
                    KH = KT // 2
                    if piece in (None, 0):
                        xh_g = xgp.tile([128, KT, 512], FP8, tag="xh", bufs=2, name="xh_g")
                        xl_g = xgp.tile([128, KT, 512], FP8, tag="xl", bufs=2, name="xl_g")
                        eps_g = egp.tile([128, GT, OS], BF16, tag="eps", bufs=2, name="eps_g")
                        groups[g] = [xh_g, xl_g, eps_g, None, None]
                    xh_g, xl_g, eps_g = groups[g][:3]
                    if piece is None:
                        nc.sync.dma_start(xh_g, xh_r[:, :, gs])
                        nc.sync.dma_start(xl_g, xl_r[:, :, gs])
                        nc.sync.dma_start(eps_g, eps_r[:, g])
                    elif piece == 0:
                        nc.sync.dma_start(xh_g[:, :KH, :], xh_r[:, :KH, gs])
                    elif piece == 1:
                        nc.sync.dma_start(xh_g[:, KH:, :], xh_r[:, KH:, gs])
                    elif piece == 2:
                        nc.sync.dma_start(xl_g[:, :KH, :], xl_r[:, :KH, gs])
                        nc.sync.dma_start(eps_g, eps_r[:, g])
                    elif piece == 3:
                        nc.sync.dma_start(xl_g[:, KH:, :], xl_r[:, KH:, gs])

                def x2_group(g):
                    xh_g = groups[g][0]
                    x2_g = xgp.tile([128, KT, 512], FP8, tag="x2", bufs=2, name="x2_g")
                    if g <= 1:
                        # prep era: Pool is busy with sc-multiplies; keep x2 off it
                        a, b_ = 20, KT
                    else:
                        a, b_ = X2_ACT, X2_ACT + X2_DVE
                    for k0_ in range(0, a, 4):
                        sl_ = slice(k0_, min(k0_ + 4, a))
                        nc.scalar.activation(x2_g[:, sl_, :], xh_g[:, sl_, :], AF.Square)
                    for k0_ in range(a, b_, 5):
                        sl_ = slice(k0_, min(k0_ + 5, b_))
                        nc.vector.tensor_tensor(x2_g[:, sl_, :], xh_g[:, sl_, :],
                                                xh_g[:, sl_, :], op=OP.mult)
                    for k0_ in range(b_, KT, 3):
                        sl_ = slice(k0_, min(k0_ + 3, KT))
                        nc.gpsimd.tensor_tensor(x2_g[:, sl_, :], xh_g[:, sl_, :],
                                                xh_g[:, sl_, :], op=OP.mult)
                    groups[g][3] = x2_g
                    outg = ogp.tile([128, GT, OS], F32, tag="og", bufs=1, name="outg")
                    groups[g][4] = outg

                def prep_unit(j, h):
                    i0 = h * IC
                    k0 = h * KC
                    js = slice(128 * j, 128 * (j + 1))
                    tn_t = wp.tile([128, IC], BF16, tag="tn", bufs=2, name="tn_t")
                    nc.sync.dma_start(tn_t, tn_d[js, i0:i0 + IC])
                    tp_t = wp.tile([128, IC], BF16, tag="tp", bufs=2, name="tp_t")
                    nc.sync.dma_start(tp_t, tp_d[js, i0:i0 + IC])
                    en = wp.tile([128, IC], BF16, tag="en", bufs=2, name="en")
                    nc.scalar.activation(en, tn_t, AF.Exp)
                    ep = wp.tile([128, IC], BF16, tag="ep", bufs=2, name="ep")
                    nc.scalar.activation(ep, tp_t, AF.Exp)
                    s1 = wp.tile([128, IC], BF16, tag="s1", bufs=1, name="s1")
                    nc.vector.scalar_tensor_tensor(s1, en, 1.0, ep,
                                                   op0=OP.add, op1=OP.add)
                    r = wp.tile([128, IC], BF16, tag="r", bufs=1, name="r")
                    with nc.allow_low_precision(reason="r in bf16 is within tolerance"):
                        nc.vector.reciprocal(r, s1)
                    d = wp.tile([128, IC], BF16, tag="d", bufs=1, name="d")
                    nc.vector.tensor_tensor(d, ep, en, op=OP.subtract)
                    diff = wp.tile([128, KC, 128], BF16, tag="diff", bufs=2, name="diff")
                    nc.vector.tensor_tensor(
                        diff, d.rearrange("p (k b) -> p k b", k=KC),
                        r.rearrange("p (k b) -> p k b", k=KC), op=OP.mult)
                    t2 = wp.tile([128, IC], BF16, tag="t2", bufs=1, name="t2")
                    nc.vector.tensor_tensor(t2, diff.rearrange("p k b -> p (k b)"),
                                            d, op=OP.mult)
                    t3 = wp.tile([128, IC], BF16, tag="t3", bufs=1, name="t3")
                    nc.vector.scalar_tensor_tensor(t3, en, -1.0, t2,
                                                   op0=OP.mult, op1=OP.add)
                    t4 = wp.tile([128, IC], BF16, tag="t4", bufs=1, name="t4")
                    nc.vector.tensor_tensor(t4, ep, t3, op=OP.subtract)
                    wvc = wp.tile([128, KC, 128], BF16, tag="wvc", bufs=2, name="wvc")
                    nc.vector.tensor_tensor(
                        wvc, t4.rearrange("p (k b) -> p k b", k=KC),
                        r.rearrange("p (k b) -> p k b", k=KC), op=OP.mult)
                    scb = scs_t[:, j, k0:k0 + KC, None].broadcast_to((128, KC, 128))
                    smw = wp.tile([128, KC, 128], BF16, tag="smw", bufs=2, name="smw")
                    nc.gpsimd.tensor_tensor(smw, diff, scb, op=OP.mult)
                    sc2b = sc2s[:, j, k0:k0 + KC, None].broadcast_to((128, KC, 128))
                    wv = wp.tile([128, KC, 128], BF16, tag="wv", bufs=2, name="wv")
                    nc.gpsimd.tensor_tensor(wv, wvc, sc2b, op=OP.mult)
                    # transpose + evacuate per half so the next unit's
                    # transposes only wait on half an evac round-trip
                    pmw = wps.tile([128, KC, 128], BF16, tag="pmw", name="pmw")
                    pwv = wps.tile([128, KC, 128], BF16, tag="pwv", name="pwv")
                    KH2 = KC // 2
                    for hf in range(2):
                        ps = slice(hf * KH2, (hf + 1) * KH2)
                        for kb in range(hf * KH2, (hf + 1) * KH2):
                            nc.tensor.transpose(pmw[:, kb, :], smw[:, kb, :], identb)
                            nc.tensor.transpose(pwv[:, kb, :], wv[:, kb, :], identb)
                        hs = slice(k0 + hf * KH2, k0 + (hf + 1) * KH2)
                        nc.scalar.activation(whT[:, hs, js], pmw[:, ps], AF.Copy)
                        nc.vector.tensor_tensor(wlT[:, hs, js], pmw[:, ps],
                                                whT[:, hs, js], op=OP.subtract)
                        nc.scalar.activation(wvT[:, hs, js], pwv[:, ps], AF.Copy)

                def mm(pt, lhs, rhs, st, sp):
                    nc.tensor.matmul(pt, lhs, rhs, start=st, stop=sp,
                                     perf_mode=DR, skip_group_check=True)

                def mu_chain(t, pmu, js):
                    g, tt = t // GT, t % GT
                    xh_g, xl_g = groups[g][0], groups[g][1]
                    ts = slice(128 * tt, 128 * (tt + 1))
                    KK = KT // 2
                    for kk in range(KK):
                        ks = slice(2 * kk, 2 * kk + 2)
                        mm(pmu[:, js], xh_g[:, ks, ts], whT[:, ks, js], kk == 0, False)
                        mm(pmu[:, js], xh_g[:, ks, ts], wlT[:, ks, js], False, False)
                        mm(pmu[:, js], xl_g[:, ks, ts], whT[:, ks, js], False, kk == KK - 1)

                def var_chain(t, pvar, js):
                    g, tt = t // GT, t % GT
                    x2_g = groups[g][3]
                    ts = slice(128 * tt, 128 * (tt + 1))
                    KK = KT // 2
                    for kk in range(KK):
                        ks = slice(2 * kk, 2 * kk + 2)
                        mm(pvar[:, js], x2_g[:, ks, ts], wvT[:, ks, js],
                           kk == 0, kk == KK - 1)

                def output_compute(t, pmu, pvar):
                    g, tt = t // GT, t % GT
                    eps_g = groups[g][2]
                    outg = groups[g][4]
                    sig = ogp.tile([128, OS], BF16, tag="sig", bufs=1, name="sig")
                    nc.scalar.activation(sig, pvar, AF.Sqrt, bias=b1e8, scale=1.0 / SV)
                    prod = ogp.tile([128, OS], BF16, tag="prod", bufs=1, name="prod")
                    nc.vector.tensor_tensor(prod, sig, eps_g[:, tt, :], op=OP.mult)
                    nc.vector.scalar_tensor_tensor(outg[:, tt, :], pmu, 1.0 / SW, prod,
                                                   op0=OP.mult, op1=OP.add)

                def out_dma(g):
                    nc.sync.dma_start(out_r[:, g], groups[g][4])

                def new_pm():
                    return ops.tile([128, OS], F32, tag="pm", bufs=3, name="pmu")

                def new_pv():
                    return ops.tile([128, OS], F32, tag="pv", bufs=3, name="pvar")

                # ---- schedule ----
                units = [(j, h) for j in range(OJ) for h in range(NH)]
                opened = {}
                full = slice(0, OS)
                for u, (j, h) in enumerate(units):
                    prep_unit(j, h)
                    if u < 4:
                        load_group(0, piece=u)
                    if u == 1:
                        x2_group(0)
                    if h == NH - 1:
                        if j == 0:
                            for t in range(N_OPEN):
                                opened[t] = [new_pm(), new_pv()]
                        js = slice(128 * j, 128 * (j + 1))
                        for t in range(N_OPEN):
                            mu_chain(t, opened[t][0], js)
                        for t in range(N_OPEN):
                            var_chain(t, opened[t][1], js)
                    if u == 13:
                        load_group(1)

                for t_ in range(N_OPEN):
                    output_compute(t_, *opened[t_])
                opened.clear()

                x2_group(1)
                pending = None
                for t in range(N_OPEN, BT):
                    g, tt = t // GT, t % GT
                    if pending is not None:
                        output_compute(*pending)
                        if pending[0] % GT == GT - 1:
                            out_dma(pending[0] // GT)
                        pending = None
                    pmu, pvar = new_pm(), new_pv()
                    var_chain(t, pvar, full)
                    mu_chain(t, pmu, full)
                    if tt == 0 and 1 <= g and g + 1 < NG:
                        load_group(g + 1)
                    if tt == 2 and 1 <= g and g + 1 < NG:
                        x2_group(g + 1)
                    pending = (t, pmu, pvar)
                output_compute(*pending)
                out_dma(NG - 1)

    nc.compile()
    return nc


def _get_nc():
    if "nc" not in _CACHE:
        _CACHE["nc"] = build()
    return _CACHE["nc"]


def kernel(x, theta_neg, theta_pos, scales_exp, eps):
    nc = _get_nc()
    np_fp8 = mybir.dt.np(FP8)
    np_bf16 = mybir.dt.np(BF16)
    xT = np.asarray(x, np.float32).T
    xh = np.ascontiguousarray(xT).astype(np_fp8)
    xl = (xT - xh.astype(np.float32)).astype(np_fp8)
    tn = np.asarray(theta_neg, np.float32).astype(np_bf16)
    tp = np.asarray(theta_pos, np.float32).astype(np_bf16)
    sc = np.asarray(scales_exp[:, ::128], np.float32)
    scs = (SW * sc).astype(np_bf16)
    scs2 = (SV * sc * sc).astype(np_bf16)
    epsb = np.asarray(eps, np.float32).astype(np_bf16)
    in_maps = []
    for j in range(N_CORES):
        sl = slice(OS * j, OS * (j + 1))
        in_maps.append({
            "xh": xh,
            "xl": xl,
            "tn": np.ascontiguousarray(tn[sl]),
            "tp": np.ascontiguousarray(tp[sl]),
            "scs": np.ascontiguousarray(scs[sl]),
            "scs2": np.ascontiguousarray(scs2[sl]),
            "eps": np.ascontiguousarray(epsb[:, sl]),
        })
    res = run_bass_kernel_spmd(nc, in_maps, core_ids=list(range(N_CORES)))
    return np.concatenate([res.results[j]["out"] for j in range(N_CORES)], axis=1)
